# revision 3
# baseline (speedup 1.0000x reference)
"""GCN (4-layer GCNConv) Trainium2 kernel — 8-core SPMD.

Strategy
--------
out = relu(^A (relu(^A (relu(^A (relu(^A X W1) W2)) W3))) W4)  with
^A = D^-1/2 (A + I) D^-1/2.  The symmetric norm dinv[src]*dinv[dst] is folded
into per-node row scalings, so the per-edge work is PURE data movement +
matmul-accumulate:

  h'_L = (dinv ⊙ o_{L-1}) @ W_L          (dense GEMM, node-sharded)
  agg[d] = sum_{e: dst(e)=d} h'_L[src(e)] (self-loops included as edges)
  o_L = relu(dinv ⊙ agg + b)             (per-node finalize)

Sharding: nodes (and their incoming edges) are split into 8 contiguous
buckets, one per NeuronCore.  Each layer: local GEMM -> AllGather of the
(fp16) feature rows -> per-core edge phase:

  * dma_gather (SWDGE) pulls h'[src] rows from HBM.  Rows are fp16 [64]
    = 128B which is below the 256B gather minimum, so indices address
    PAIRS of rows (idx = row>>1, elem 128 fp16 = 256B) and edges are
    grouped host-side by row parity; odd-parity groups gather from a
    64-element-shifted view so the payload is always elems [0:64].
  * edges are sorted by dst window (128 dst nodes); a one-hot fp16 matrix
    (built on DVE with a single is_equal tensor_tensor per chunk) maps each
    128-edge tile onto its dst slots, and the tensor engine accumulates
      psum[dst_slot, feat] += onehot.T @ msg
    into a per-window PSUM tile.  No scatter DMA, no per-edge vector math.

Host-side preprocessing (numpy, outside the timed kernel): degree/dinv
computation, bucketing + window sorting of edges, parity split, padding to
uniform shapes across cores (SPMD shares one program).
"""

import math
import os
import sys
from contextlib import ExitStack
from dataclasses import dataclass, field

import numpy as np

for _p in ("/opt/trn_rl_repo", "/root/.axon_site/_ro/trn_rl_repo"):
    if os.path.isdir(_p) and _p not in sys.path:
        sys.path.append(_p)

import concourse.bacc as bacc
import concourse.bass as bass
import concourse.tile as tile
from concourse import mybir
from concourse.bass_utils import run_bass_kernel_spmd

F32 = mybir.dt.float32
F16 = mybir.dt.float16
I16 = mybir.dt.int16

NPF16 = np.float16
SENTINEL = 255.0  # dst-in-window value for pad slots -> all-zero one-hot column


@dataclass
class Cfg:
    N: int = 50000
    IN: int = 128
    HID: int = 64
    C: int = 8
    WIN: int = 64        # dst window width (one-hot columns); 128 or 64
    SG: int = 2          # psum-pairs (128 dst rows) per gather super-group
    GEMM_CHUNK: int = 512

    @property
    def NL(self):
        return self.N // self.C

    @property
    def NWIN(self):       # pairs of 128 dst rows (psum tiles / staging cols)
        return (self.NL + 127) // 128

    @property
    def NWINW(self):      # one-hot windows
        return (self.NL + self.WIN - 1) // self.WIN

    @property
    def WPP(self):        # windows per psum pair-tile
        return 128 // self.WIN

    @property
    def NLP(self):
        return self.NWIN * 128

    @property
    def NROW(self):
        return self.C * self.NLP


@dataclass
class TileRec:
    w: int          # window within supergroup's window list (global window id)
    start: bool
    stop: bool


@dataclass
class ChunkRec:
    par: int
    t0: int         # first global tile index
    tiles: list = field(default_factory=list)  # list[TileRec]


def _build_layout(cfg: Cfg, counts: np.ndarray):
    """counts: [C, NWINW, 2] raw edge counts per (core, window, parity).
    Returns (chunks, T_total, S_total, group_caps[NWINW,2]) shared by all
    cores.  Supergroups cover SG psum-pairs (= SG*WPP windows) so a pair's
    windows always complete within one supergroup."""
    caps = counts.max(axis=0)  # [NWINW, 2]
    caps = ((caps + 127) // 128) * 128
    ntiles = caps // 128  # [NWINW, 2]

    chunks: list[ChunkRec] = []
    t = 0
    first_seen = set()
    last_par = {}
    for w in range(cfg.NWINW):
        if ntiles[w, 1] > 0:
            last_par[w] = 1
        elif ntiles[w, 0] > 0:
            last_par[w] = 0
    sgw = cfg.SG * cfg.WPP
    for sg0 in range(0, cfg.NWINW, sgw):
        ws = range(sg0, min(sg0 + sgw, cfg.NWINW))
        for par in (0, 1):
            ch = ChunkRec(par=par, t0=t)
            for w in ws:
                nt = int(ntiles[w, par])
                for j in range(nt):
                    start = w not in first_seen
                    first_seen.add(w)
                    stop = (par == last_par.get(w)) and (j == nt - 1)
                    ch.tiles.append(TileRec(w=w, start=start, stop=stop))
                    t += 1
            if ch.tiles:
                chunks.append(ch)
    T_total = t
    S_total = T_total * 128
    return chunks, T_total, S_total, caps


def host_prep(cfg: Cfg, x: np.ndarray, edge_index: np.ndarray,
              Ws: list, bs: list):
    """Returns (in_maps, chunks, T_total, S_total)."""
    N, NL, NLP, NWIN, C = cfg.N, cfg.NL, cfg.NLP, cfg.NWIN, cfg.C
    src = np.concatenate([np.asarray(edge_index[0], np.int64), np.arange(N)])
    dst = np.concatenate([np.asarray(edge_index[1], np.int64), np.arange(N)])
    deg = np.bincount(dst, minlength=N).astype(np.float32)
    dinv = (1.0 / np.sqrt(deg)).astype(np.float32)

    grow = (src // NL) * NLP + (src % NL)     # global padded row of each edge src
    par = (grow & 1).astype(np.int64)
    sidx = (grow >> 1).astype(np.int64)

    NWINW = cfg.NWINW
    bucket = dst // NL
    wloc = (dst % NL) // cfg.WIN
    dwin = (dst % NL) % cfg.WIN

    counts = np.zeros((C, NWINW, 2), np.int64)
    per_core = []
    for k in range(C):
        m = bucket == k
        g = wloc[m] * 2 + par[m]
        counts[k] = np.bincount(g, minlength=NWINW * 2).reshape(NWINW, 2)
        order = np.argsort(g, kind="stable")
        per_core.append((g[order], sidx[m][order], dwin[m][order]))

    chunks, T_total, S_total, caps = _build_layout(cfg, counts)

    # slot offsets per (w, par) group, following chunk order
    offs = np.zeros((NWINW, 2), np.int64)
    pos = 0
    sgw = cfg.SG * cfg.WPP
    for sg0 in range(0, NWINW, sgw):
        ws = range(sg0, min(sg0 + sgw, NWINW))
        for p in (0, 1):
            for w in ws:
                offs[w, p] = pos
                pos += caps[w, p]
    assert pos == S_total

    xp = np.asarray(x, np.float32) * dinv[:, None]

    in_maps = []
    for k in range(C):
        g, si, dw = per_core[k]
        gidx = np.zeros(S_total, np.int16)
        dstw = np.full(S_total, SENTINEL, NPF16)
        # starts of groups in the sorted per-core arrays
        gcnt = counts[k].reshape(-1)
        gstart = np.zeros(NWINW * 2, np.int64)
        gstart[1:] = np.cumsum(gcnt)[:-1]
        for w in range(NWINW):
            for p in (0, 1):
                n = int(counts[k, w, p])
                if n == 0:
                    continue
                s = int(gstart[w * 2 + p])
                o = int(offs[w, p])
                gidx[o:o + n] = si[s:s + n]
                dstw[o:o + n] = dw[s:s + n].astype(NPF16)
        gimg = np.tile(gidx.reshape(-1, 16).T, (8, 1))          # [128, S/16]
        dimg = dstw.reshape(T_total, 128).T.copy()              # [128, T]

        dinvw = np.zeros((128, NWIN), np.float32)
        for w in range(NWIN):
            nvalid = min(128, NL - w * 128)
            dinvw[:nvalid, w] = dinv[k * NL + w * 128: k * NL + w * 128 + nvalid]

        xT = np.zeros((cfg.IN, NLP), np.float32)
        xT[:, :NL] = xp[k * NL:(k + 1) * NL].T

        iota = np.broadcast_to(np.arange(128, dtype=NPF16), (128, 128)).copy()
        idf32 = np.eye(128, dtype=np.float32)

        im = {"xT": xT, "gidx": gimg, "dstw": dimg, "dinvw": dinvw,
              "iota": iota, "idf32": idf32}
        for i, (W, b) in enumerate(zip(Ws, bs), start=1):
            im[f"w{i}"] = np.asarray(W, np.float32)
            im[f"bb{i}"] = np.broadcast_to(np.asarray(b, np.float32),
                                           (128, cfg.HID)).copy()
        in_maps.append(im)
    return in_maps, chunks, T_total, S_total


def build_kernel(cfg: Cfg, chunks, T_total: int, S_total: int, debug=False):
    NL, NLP, NWIN, NROW, HID, IN = (cfg.NL, cfg.NLP, cfg.NWIN, cfg.NROW,
                                    cfg.HID, cfg.IN)
    nc = bacc.Bacc("TRN2", target_bir_lowering=False, debug=debug,
                   num_devices=cfg.C, num_swdge_queues=4)

    xT_d = nc.dram_tensor("xT", [IN, NLP], F32, kind="ExternalInput")
    gidx_d = nc.dram_tensor("gidx", [128, S_total // 16], I16, kind="ExternalInput")
    dstw_d = nc.dram_tensor("dstw", [128, T_total], F16, kind="ExternalInput")
    dinvw_d = nc.dram_tensor("dinvw", [128, NWIN], F32, kind="ExternalInput")
    iota_d = nc.dram_tensor("iota", [128, 128], F16, kind="ExternalInput")
    idf32_d = nc.dram_tensor("idf32", [128, 128], F32, kind="ExternalInput")
    w_d, bb_d = {}, {}
    for L in range(1, 5):
        KD = IN if L == 1 else HID
        w_d[L] = nc.dram_tensor(f"w{L}", [KD, HID], F32, kind="ExternalInput")
        bb_d[L] = nc.dram_tensor(f"bb{L}", [128, HID], F32, kind="ExternalInput")
    out_d = nc.dram_tensor("out", [NLP, HID], F32, kind="ExternalOutput")

    h_loc, h_full = {}, {}
    for L in range(1, 5):
        h_loc[L] = nc.dram_tensor(f"hloc{L}", [NLP, HID], F16)
        # +2 pad rows so the pair-packed odd-parity gather view never reads
        # past the tensor; they are zeroed once at startup.
        h_full[L] = nc.dram_tensor(f"hfull{L}", [NROW + 2, HID], F16,
                                   addr_space="Shared")

    groups = [list(range(cfg.C))]

    with tile.TileContext(nc) as tc, ExitStack() as ctx:
        cpool = ctx.enter_context(tc.tile_pool(name="consts", bufs=1))
        spool = ctx.enter_context(tc.tile_pool(name="stage", bufs=1))
        msg_pool = ctx.enter_context(tc.tile_pool(name="msg", bufs=3))
        oh_pool = ctx.enter_context(tc.tile_pool(name="oh", bufs=3))
        fin_pool = ctx.enter_context(tc.tile_pool(name="fin", bufs=4))
        ps_win = ctx.enter_context(tc.tile_pool(name="ps_win", bufs=4, space="PSUM"))
        ps_gemm = ctx.enter_context(tc.tile_pool(name="ps_gemm", bufs=2, space="PSUM"))
        ps_tr = ctx.enter_context(tc.tile_pool(name="ps_tr", bufs=2, space="PSUM"))
        # PSUM bank budget: ps_win 4 + ps_gemm 2 + ps_tr 2 (one shared tag) = 8

        def cload(dram, shape, dtype, tag):
            t = cpool.tile(shape, dtype, tag=tag)
            nc.sync.dma_start(t[:], dram[:])
            return t

        xT_s = cload(xT_d, [IN, NLP], F32, "xT")
        gidx_s = cload(gidx_d, [128, S_total // 16], I16, "gidx")
        dstw_s = cload(dstw_d, [128, T_total], F16, "dstw")
        dinvw_s = cload(dinvw_d, [128, NWIN], F32, "dinvw")
        iota_s = cload(iota_d, [128, 128], F16, "iota")
        idf32_s = cload(idf32_d, [128, 128], F32, "idf32")
        w_s = {L: cload(w_d[L], w_d[L].shape, F32, f"w{L}") for L in range(1, 5)}
        bb_s = {L: cload(bb_d[L], [128, HID], F32, f"bb{L}") for L in range(1, 5)}

        hT_s = spool.tile([HID, NLP], F32, tag="hT")
        oT_s = spool.tile([HID, NLP], F32, tag="oT")
        hstg = spool.tile([128, NWIN, HID], F16, tag="hstg")
        ostg = spool.tile([128, NWIN, HID], F32, tag="ostg")

        # zero the +2 trailing pad rows of every h_full (NaN-safety for the
        # pair-packed gather: junk*0 would still poison psum if junk is NaN)
        zrow = cpool.tile([2, HID], F16, tag="zrow")
        nc.vector.memset(zrow[:], 0)
        for L in range(1, 5):
            nc.sync.dma_start(h_full[L][NROW:NROW + 2], zrow[:])

        NC2 = NROW // 2
        abl_layers = int(os.environ.get("ABL_LAYERS", "4"))
        abl_nogather = bool(int(os.environ.get("ABL_NOGATHER", "0")))
        abl_nocc = bool(int(os.environ.get("ABL_NOCC", "0")))
        abl_repeat = int(os.environ.get("ABL_REPEAT", "1"))
        for L in [l for _ in range(abl_repeat)
                  for l in range(1, abl_layers + 1)]:
            KD = IN if L == 1 else HID
            movin = xT_s if L == 1 else oT_s

            # ---- GEMM: hT = W.T @ movin  -> [HID, NLP] (f32) ----
            for c0 in range(0, NLP, cfg.GEMM_CHUNK):
                cw = min(cfg.GEMM_CHUNK, NLP - c0)
                pg = ps_gemm.tile([HID, cfg.GEMM_CHUNK], F32, tag="pg")
                nc.tensor.matmul(pg[:, :cw], w_s[L][:KD, :], movin[:KD, c0:c0 + cw],
                                 start=True, stop=True)
                nc.scalar.copy(hT_s[:, c0:c0 + cw], pg[:, :cw])

            # ---- transpose hT -> fp16 rows, stage, store, allgather ----
            for w in range(NWIN):
                pt = ps_tr.tile([128, 128], F32, tag="pt")
                nc.tensor.transpose(pt[:, :HID],
                                    hT_s[:, w * 128:(w + 1) * 128],
                                    idf32_s[:HID, :HID])
                nc.scalar.copy(hstg[:, w, :], pt[:, :HID])
            nc.sync.dma_start(
                h_loc[L][:].rearrange("(w p) f -> p w f", p=128), hstg[:])
            if abl_nocc:
                nc.sync.dma_start(h_full[L][:NLP], h_loc[L][:])
            else:
                nc.gpsimd.collective_compute(
                    "AllGather", mybir.AluOpType.bypass, replica_groups=groups,
                    ins=[h_loc[L][:]], outs=[h_full[L][:NROW]])

            flat = h_full[L][:].flatten()
            view = [
                flat[0:NC2 * 128].rearrange("(n e) -> n e", e=128),
                flat[64:64 + NC2 * 128].rearrange("(n e) -> n e", e=128),
            ]

            # ---- edge phase ----
            WINW, WPP = cfg.WIN, cfg.WPP
            pair_nstop = {}
            for ch in chunks:
                for tl in ch.tiles:
                    if tl.stop:
                        p = (tl.w * WINW) // 128
                        pair_nstop[p] = pair_nstop.get(p, 0) + 1
            win_ps, pair_left = {}, {}
            for ci, ch in enumerate(chunks):
                nt = len(ch.tiles)
                msg = msg_pool.tile([128, nt, 128], F16, tag="msg")
                if abl_nogather:
                    nc.vector.memset(msg[:], 0)
                else:
                    nc.gpsimd.dma_gather(
                        msg[:], view[ch.par],
                        gidx_s[:, ch.t0 * 8:(ch.t0 + nt) * 8],
                        nt * 128, nt * 128, 128, single_packet=False,
                        queue_num=ci % 4)
                oh = oh_pool.tile([128, nt, WINW], F16, tag="oh")
                nc.vector.tensor_tensor(
                    oh[:],
                    iota_s[:, :WINW].unsqueeze(1).broadcast_to((128, nt, WINW)),
                    dstw_s[:, ch.t0:ch.t0 + nt].unsqueeze(2)
                        .broadcast_to((128, nt, WINW)),
                    mybir.AluOpType.is_equal)
                for j, tl in enumerate(ch.tiles):
                    pair = (tl.w * WINW) // 128
                    soff = (tl.w * WINW) % 128
                    if pair not in win_ps:
                        win_ps[pair] = ps_win.tile([128, HID], F32, tag="pw",
                                                   name=f"pw{L}_{pair}")
                        pair_left[pair] = pair_nstop[pair]
                    nc.tensor.matmul(win_ps[pair][soff:soff + WINW, :],
                                     oh[:, j, :], msg[:, j, :HID],
                                     start=tl.start, stop=tl.stop,
                                     skip_group_check=True)
                    if tl.stop:
                        pair_left[pair] -= 1
                        if pair_left[pair] > 0:
                            continue
                        pw = win_ps.pop(pair)
                        dv = dinvw_s[:, pair:pair + 1]
                        if L < 4:
                            tmp = fin_pool.tile([128, HID], F32, tag="fin")
                            nc.vector.scalar_tensor_tensor(
                                tmp[:], pw[:], dv, bb_s[L][:],
                                mybir.AluOpType.mult, mybir.AluOpType.add)
                            nc.scalar.activation(
                                ostg[:, pair, :], tmp[:],
                                mybir.ActivationFunctionType.Relu, scale=dv)
                        else:
                            nc.vector.scalar_tensor_tensor(
                                ostg[:, pair, :], pw[:], dv, bb_s[L][:],
                                mybir.AluOpType.mult, mybir.AluOpType.add)

            if L < 4:
                # ---- oT = transpose(ostg) for next GEMM ----
                for w in range(NWIN):
                    pt = ps_tr.tile([128, 128], F32, tag="pt")
                    nc.tensor.transpose(pt[:HID, :],
                                        ostg[:, w, :], idf32_s[:])
                    nc.scalar.copy(oT_s[:, w * 128:(w + 1) * 128], pt[:HID, :])
            else:
                nc.sync.dma_start(
                    out_d[:].rearrange("(w p) f -> p w f", p=128), ostg[:])

    nc.compile()
    return nc


# ---------------------------------------------------------------------------

_CACHE = {}


def _get_kernel(cfg, x, edge_index, Ws, bs):
    in_maps, chunks, T_total, S_total = host_prep(cfg, x, edge_index, Ws, bs)
    key = (cfg.N, cfg.IN, T_total, S_total,
           tuple((c.par, c.t0, len(c.tiles)) for c in chunks))
    if key not in _CACHE:
        _CACHE[key] = build_kernel(cfg, chunks, T_total, S_total)
    return _CACHE[key], in_maps


def run(cfg: Cfg, x, edge_index, Ws, bs, trace=False):
    nc, in_maps = _get_kernel(cfg, x, edge_index, Ws, bs)
    res = run_bass_kernel_spmd(nc, in_maps, list(range(cfg.C)), trace=trace)
    out = np.concatenate([res.results[k]["out"][:cfg.NL]
                          for k in range(cfg.C)], axis=0)
    return out, res


def kernel(x, edge_index, W1, b1, W2, b2, W3, b3, W4, b4):
    cfg = Cfg(N=x.shape[0], IN=x.shape[1])
    out, _ = run(cfg, np.asarray(x), np.asarray(edge_index),
                 [W1, W2, W3, W4], [b1, b2, b3, b4])
    return out.astype(np.float32)



# revision 4
# speedup vs baseline: 1.2788x; 1.2788x over previous
"""GCN (4-layer GCNConv) Trainium2 kernel — 8-core SPMD.

Strategy
--------
out = relu(^A (relu(^A (relu(^A (relu(^A X W1) W2)) W3))) W4)  with
^A = D^-1/2 (A + I) D^-1/2.  The symmetric norm dinv[src]*dinv[dst] is folded
into per-node row scalings, so the per-edge work is PURE data movement +
matmul-accumulate:

  h'_L = (dinv ⊙ o_{L-1}) @ W_L          (dense GEMM, node-sharded)
  agg[d] = sum_{e: dst(e)=d} h'_L[src(e)] (self-loops included as edges)
  o_L = relu(dinv ⊙ agg + b)             (per-node finalize)

Sharding: nodes (and their incoming edges) are split into 8 contiguous
buckets, one per NeuronCore.  Each layer: local GEMM -> AllGather of the
(fp16) feature rows -> per-core edge phase:

  * dma_gather (SWDGE) pulls h'[src] rows from HBM.  Rows are fp16 [64]
    = 128B which is below the 256B gather minimum, so indices address
    PAIRS of rows (idx = row>>1, elem 128 fp16 = 256B) and edges are
    grouped host-side by row parity; odd-parity groups gather from a
    64-element-shifted view so the payload is always elems [0:64].
  * edges are sorted by dst window (128 dst nodes); a one-hot fp16 matrix
    (built on DVE with a single is_equal tensor_tensor per chunk) maps each
    128-edge tile onto its dst slots, and the tensor engine accumulates
      psum[dst_slot, feat] += onehot.T @ msg
    into a per-window PSUM tile.  No scatter DMA, no per-edge vector math.

Host-side preprocessing (numpy, outside the timed kernel): degree/dinv
computation, bucketing + window sorting of edges, parity split, padding to
uniform shapes across cores (SPMD shares one program).
"""

import math
import os
import sys
from contextlib import ExitStack
from dataclasses import dataclass, field

import numpy as np

for _p in ("/opt/trn_rl_repo", "/root/.axon_site/_ro/trn_rl_repo"):
    if os.path.isdir(_p) and _p not in sys.path:
        sys.path.append(_p)

import concourse.bacc as bacc
import concourse.bass as bass
import concourse.tile as tile
from concourse import mybir
from concourse.bass_utils import run_bass_kernel_spmd

F32 = mybir.dt.float32
F16 = mybir.dt.float16
I16 = mybir.dt.int16

NPF16 = np.float16
SENTINEL = 255.0  # dst-in-window value for pad slots -> all-zero one-hot column


@dataclass
class Cfg:
    N: int = 50000
    IN: int = 128
    HID: int = 64
    C: int = 8
    WIN: int = 64        # dst window width (one-hot columns); 128 or 64
    SG: int = 2          # psum-pairs (128 dst rows) per gather super-group
    GEMM_CHUNK: int = 512

    @property
    def NL(self):
        return self.N // self.C

    @property
    def NWIN(self):       # pairs of 128 dst rows (psum tiles / staging cols)
        return (self.NL + 127) // 128

    @property
    def NWINW(self):      # one-hot windows
        return (self.NL + self.WIN - 1) // self.WIN

    @property
    def WPP(self):        # windows per psum pair-tile
        return 128 // self.WIN

    @property
    def NLP(self):
        return self.NWIN * 128

    @property
    def NROW(self):
        return self.C * self.NLP


@dataclass
class TileRec:
    w: int          # window within supergroup's window list (global window id)
    start: bool
    stop: bool


@dataclass
class ChunkRec:
    par: int
    t0: int         # first global tile index
    tiles: list = field(default_factory=list)  # list[TileRec]


def _build_layout(cfg: Cfg, counts: np.ndarray):
    """counts: [C, NWINW, 2] raw edge counts per (core, window, parity).
    Returns (chunks, T_total, S_total, group_caps[NWINW,2]) shared by all
    cores.  Supergroups cover SG psum-pairs (= SG*WPP windows) so a pair's
    windows always complete within one supergroup."""
    caps = counts.max(axis=0)  # [NWINW, 2]
    caps = ((caps + 127) // 128) * 128
    ntiles = caps // 128  # [NWINW, 2]

    chunks: list[ChunkRec] = []
    t = 0
    first_seen = set()
    last_par = {}
    for w in range(cfg.NWINW):
        if ntiles[w, 1] > 0:
            last_par[w] = 1
        elif ntiles[w, 0] > 0:
            last_par[w] = 0
    sgw = cfg.SG * cfg.WPP
    for sg0 in range(0, cfg.NWINW, sgw):
        ws = range(sg0, min(sg0 + sgw, cfg.NWINW))
        for par in (0, 1):
            ch = ChunkRec(par=par, t0=t)
            for w in ws:
                nt = int(ntiles[w, par])
                for j in range(nt):
                    start = w not in first_seen
                    first_seen.add(w)
                    stop = (par == last_par.get(w)) and (j == nt - 1)
                    ch.tiles.append(TileRec(w=w, start=start, stop=stop))
                    t += 1
            if ch.tiles:
                chunks.append(ch)
    T_total = t
    S_total = T_total * 128
    return chunks, T_total, S_total, caps


def host_prep(cfg: Cfg, x: np.ndarray, edge_index: np.ndarray,
              Ws: list, bs: list):
    """Returns (in_maps, chunks, T_total, S_total)."""
    N, NL, NLP, NWIN, C = cfg.N, cfg.NL, cfg.NLP, cfg.NWIN, cfg.C
    src = np.concatenate([np.asarray(edge_index[0], np.int64), np.arange(N)])
    dst = np.concatenate([np.asarray(edge_index[1], np.int64), np.arange(N)])
    deg = np.bincount(dst, minlength=N).astype(np.float32)
    dinv = (1.0 / np.sqrt(deg)).astype(np.float32)

    grow = (src // NL) * NLP + (src % NL)     # global padded row of each edge src
    par = (grow & 1).astype(np.int64)
    sidx = (grow >> 1).astype(np.int64)

    NWINW = cfg.NWINW
    bucket = dst // NL
    wloc = (dst % NL) // cfg.WIN
    dwin = (dst % NL) % cfg.WIN

    counts = np.zeros((C, NWINW, 2), np.int64)
    per_core = []
    for k in range(C):
        m = bucket == k
        g = wloc[m] * 2 + par[m]
        counts[k] = np.bincount(g, minlength=NWINW * 2).reshape(NWINW, 2)
        order = np.argsort(g, kind="stable")
        per_core.append((g[order], sidx[m][order], dwin[m][order]))

    chunks, T_total, S_total, caps = _build_layout(cfg, counts)

    # slot offsets per (w, par) group, following chunk order
    offs = np.zeros((NWINW, 2), np.int64)
    pos = 0
    sgw = cfg.SG * cfg.WPP
    for sg0 in range(0, NWINW, sgw):
        ws = range(sg0, min(sg0 + sgw, NWINW))
        for p in (0, 1):
            for w in ws:
                offs[w, p] = pos
                pos += caps[w, p]
    assert pos == S_total

    xp = np.asarray(x, np.float32) * dinv[:, None]

    in_maps = []
    for k in range(C):
        g, si, dw = per_core[k]
        gidx = np.zeros(S_total, np.int16)
        dstw = np.full(S_total, SENTINEL, NPF16)
        # starts of groups in the sorted per-core arrays
        gcnt = counts[k].reshape(-1)
        gstart = np.zeros(NWINW * 2, np.int64)
        gstart[1:] = np.cumsum(gcnt)[:-1]
        for w in range(NWINW):
            for p in (0, 1):
                n = int(counts[k, w, p])
                if n == 0:
                    continue
                s = int(gstart[w * 2 + p])
                o = int(offs[w, p])
                gidx[o:o + n] = si[s:s + n]
                dstw[o:o + n] = dw[s:s + n].astype(NPF16)
        gimg = np.tile(gidx.reshape(-1, 16).T, (8, 1))          # [128, S/16]
        dimg = dstw.reshape(T_total, 128).T.copy()              # [128, T]

        dinvw = np.zeros((128, NWIN), np.float32)
        for w in range(NWIN):
            nvalid = min(128, NL - w * 128)
            dinvw[:nvalid, w] = dinv[k * NL + w * 128: k * NL + w * 128 + nvalid]

        xT = np.zeros((cfg.IN, NLP), np.float32)
        xT[:, :NL] = xp[k * NL:(k + 1) * NL].T

        iota = np.broadcast_to(np.arange(128, dtype=NPF16), (128, 128)).copy()
        idf32 = np.eye(128, dtype=np.float32)

        im = {"xT": xT, "gidx": gimg, "dstw": dimg, "dinvw": dinvw,
              "iota": iota, "idf32": idf32}
        for i, (W, b) in enumerate(zip(Ws, bs), start=1):
            im[f"w{i}"] = np.asarray(W, np.float32)
            im[f"bb{i}"] = np.broadcast_to(np.asarray(b, np.float32),
                                           (128, cfg.HID)).copy()
        in_maps.append(im)
    return in_maps, chunks, T_total, S_total


def build_kernel(cfg: Cfg, chunks, T_total: int, S_total: int, debug=False):
    NL, NLP, NWIN, NROW, HID, IN = (cfg.NL, cfg.NLP, cfg.NWIN, cfg.NROW,
                                    cfg.HID, cfg.IN)
    nc = bacc.Bacc("TRN2", target_bir_lowering=False, debug=debug,
                   num_devices=cfg.C, num_swdge_queues=4)

    xT_d = nc.dram_tensor("xT", [IN, NLP], F32, kind="ExternalInput")
    gidx_d = nc.dram_tensor("gidx", [128, S_total // 16], I16, kind="ExternalInput")
    dstw_d = nc.dram_tensor("dstw", [128, T_total], F16, kind="ExternalInput")
    dinvw_d = nc.dram_tensor("dinvw", [128, NWIN], F32, kind="ExternalInput")
    iota_d = nc.dram_tensor("iota", [128, 128], F16, kind="ExternalInput")
    idf32_d = nc.dram_tensor("idf32", [128, 128], F32, kind="ExternalInput")
    w_d, bb_d = {}, {}
    for L in range(1, 5):
        KD = IN if L == 1 else HID
        w_d[L] = nc.dram_tensor(f"w{L}", [KD, HID], F32, kind="ExternalInput")
        bb_d[L] = nc.dram_tensor(f"bb{L}", [128, HID], F32, kind="ExternalInput")
    out_d = nc.dram_tensor("out", [NLP, HID], F32, kind="ExternalOutput")

    h_loc, h_full = {}, {}
    for L in range(1, 5):
        h_loc[L] = nc.dram_tensor(f"hloc{L}", [NLP, HID], F16)
        # +2 pad rows so the pair-packed odd-parity gather view never reads
        # past the tensor; they are zeroed once at startup.
        h_full[L] = nc.dram_tensor(f"hfull{L}", [NROW + 2, HID], F16,
                                   addr_space="Shared")

    groups = [list(range(cfg.C))]

    with tile.TileContext(nc) as tc, ExitStack() as ctx:
        cpool = ctx.enter_context(tc.tile_pool(name="consts", bufs=1))
        spool = ctx.enter_context(tc.tile_pool(name="stage", bufs=1))
        msg_pool = ctx.enter_context(tc.tile_pool(name="msg", bufs=5))
        oh_pool = ctx.enter_context(tc.tile_pool(name="oh", bufs=4))
        fin_pool = ctx.enter_context(tc.tile_pool(name="fin", bufs=4))
        ps_win = ctx.enter_context(tc.tile_pool(name="ps_win", bufs=4, space="PSUM"))
        ps_gemm = ctx.enter_context(tc.tile_pool(name="ps_gemm", bufs=2, space="PSUM"))
        ps_tr = ctx.enter_context(tc.tile_pool(name="ps_tr", bufs=2, space="PSUM"))
        # PSUM bank budget: ps_win 4 + ps_gemm 2 + ps_tr 2 (one shared tag) = 8

        def cload(dram, shape, dtype, tag):
            t = cpool.tile(shape, dtype, tag=tag)
            nc.sync.dma_start(t[:], dram[:])
            return t

        xT_s = cload(xT_d, [IN, NLP], F32, "xT")
        gidx_s = cload(gidx_d, [128, S_total // 16], I16, "gidx")
        dstw_s = cload(dstw_d, [128, T_total], F16, "dstw")
        dinvw_s = cload(dinvw_d, [128, NWIN], F32, "dinvw")
        iota_s = cload(iota_d, [128, 128], F16, "iota")
        idf32_s = cload(idf32_d, [128, 128], F32, "idf32")
        w_s = {L: cload(w_d[L], w_d[L].shape, F32, f"w{L}") for L in range(1, 5)}
        bb_s = {L: cload(bb_d[L], [128, HID], F32, f"bb{L}") for L in range(1, 5)}

        hT_s = spool.tile([HID, NLP], F32, tag="hT")
        oT_s = spool.tile([HID, NLP], F32, tag="oT")
        hstg = spool.tile([128, NWIN, HID], F16, tag="hstg")
        ostg = spool.tile([128, NWIN, HID], F32, tag="ostg")

        # zero the +2 trailing pad rows of every h_full (NaN-safety for the
        # pair-packed gather: junk*0 would still poison psum if junk is NaN)
        zrow = cpool.tile([2, HID], F16, tag="zrow")
        nc.vector.memset(zrow[:], 0)
        for L in range(1, 5):
            nc.sync.dma_start(h_full[L][NROW:NROW + 2], zrow[:])

        NC2 = NROW // 2
        abl_layers = int(os.environ.get("ABL_LAYERS", "4"))
        abl_nogather = bool(int(os.environ.get("ABL_NOGATHER", "0")))
        abl_nocc = bool(int(os.environ.get("ABL_NOCC", "0")))
        abl_repeat = int(os.environ.get("ABL_REPEAT", "1"))
        for L in [l for _ in range(abl_repeat)
                  for l in range(1, abl_layers + 1)]:
            KD = IN if L == 1 else HID
            movin = xT_s if L == 1 else oT_s

            # ---- GEMM: hT = W.T @ movin  -> [HID, NLP] (f32) ----
            for c0 in range(0, NLP, cfg.GEMM_CHUNK):
                cw = min(cfg.GEMM_CHUNK, NLP - c0)
                pg = ps_gemm.tile([HID, cfg.GEMM_CHUNK], F32, tag="pg")
                nc.tensor.matmul(pg[:, :cw], w_s[L][:KD, :], movin[:KD, c0:c0 + cw],
                                 start=True, stop=True)
                nc.scalar.copy(hT_s[:, c0:c0 + cw], pg[:, :cw])

            # ---- transpose hT -> fp16 rows, stage, store, allgather ----
            for w in range(NWIN):
                pt = ps_tr.tile([128, 128], F32, tag="pt")
                nc.tensor.transpose(pt[:, :HID],
                                    hT_s[:, w * 128:(w + 1) * 128],
                                    idf32_s[:HID, :HID])
                nc.scalar.copy(hstg[:, w, :], pt[:, :HID])
            nc.sync.dma_start(
                h_loc[L][:].rearrange("(w p) f -> p w f", p=128), hstg[:])
            if abl_nocc:
                nc.sync.dma_start(h_full[L][:NLP], h_loc[L][:])
            else:
                nc.gpsimd.collective_compute(
                    "AllGather", mybir.AluOpType.bypass, replica_groups=groups,
                    ins=[h_loc[L][:]], outs=[h_full[L][:NROW]])

            flat = h_full[L][:].flatten()
            view = [
                flat[0:NC2 * 128].rearrange("(n e) -> n e", e=128),
                flat[64:64 + NC2 * 128].rearrange("(n e) -> n e", e=128),
            ]

            # ---- edge phase ----
            WINW, WPP = cfg.WIN, cfg.WPP
            pair_nstop = {}
            for ch in chunks:
                for tl in ch.tiles:
                    if tl.stop:
                        p = (tl.w * WINW) // 128
                        pair_nstop[p] = pair_nstop.get(p, 0) + 1
            win_ps, pair_left = {}, {}
            for ci, ch in enumerate(chunks):
                nt = len(ch.tiles)
                msg = msg_pool.tile([128, nt, 128], F16, tag="msg")
                if abl_nogather:
                    nc.vector.memset(msg[:], 0)
                else:
                    nc.gpsimd.dma_gather(
                        msg[:], view[ch.par],
                        gidx_s[:, ch.t0 * 8:(ch.t0 + nt) * 8],
                        nt * 128, nt * 128, 128, single_packet=False,
                        queue_num=ci % 4)
                oh = oh_pool.tile([128, nt, WINW], F16, tag="oh")
                nc.vector.tensor_tensor(
                    oh[:],
                    iota_s[:, :WINW].unsqueeze(1).broadcast_to((128, nt, WINW)),
                    dstw_s[:, ch.t0:ch.t0 + nt].unsqueeze(2)
                        .broadcast_to((128, nt, WINW)),
                    mybir.AluOpType.is_equal)
                for j, tl in enumerate(ch.tiles):
                    pair = (tl.w * WINW) // 128
                    soff = (tl.w * WINW) % 128
                    if pair not in win_ps:
                        win_ps[pair] = ps_win.tile([128, HID], F32, tag="pw",
                                                   name=f"pw{L}_{pair}")
                        pair_left[pair] = pair_nstop[pair]
                    nc.tensor.matmul(win_ps[pair][soff:soff + WINW, :],
                                     oh[:, j, :], msg[:, j, :HID],
                                     start=tl.start, stop=tl.stop,
                                     skip_group_check=True)
                    if tl.stop:
                        pair_left[pair] -= 1
                        if pair_left[pair] > 0:
                            continue
                        pw = win_ps.pop(pair)
                        dv = dinvw_s[:, pair:pair + 1]
                        if L < 4:
                            tmp = fin_pool.tile([128, HID], F32, tag="fin")
                            nc.vector.scalar_tensor_tensor(
                                tmp[:], pw[:], dv, bb_s[L][:],
                                mybir.AluOpType.mult, mybir.AluOpType.add)
                            nc.scalar.activation(
                                ostg[:, pair, :], tmp[:],
                                mybir.ActivationFunctionType.Relu, scale=dv)
                        else:
                            nc.vector.scalar_tensor_tensor(
                                ostg[:, pair, :], pw[:], dv, bb_s[L][:],
                                mybir.AluOpType.mult, mybir.AluOpType.add)

            if L < 4:
                # ---- oT = transpose(ostg) for next GEMM ----
                for w in range(NWIN):
                    pt = ps_tr.tile([128, 128], F32, tag="pt")
                    nc.tensor.transpose(pt[:HID, :],
                                        ostg[:, w, :], idf32_s[:])
                    nc.scalar.copy(oT_s[:, w * 128:(w + 1) * 128], pt[:HID, :])
            else:
                nc.sync.dma_start(
                    out_d[:].rearrange("(w p) f -> p w f", p=128), ostg[:])

    nc.compile()
    return nc


# ---------------------------------------------------------------------------

_CACHE = {}


def _get_kernel(cfg, x, edge_index, Ws, bs):
    in_maps, chunks, T_total, S_total = host_prep(cfg, x, edge_index, Ws, bs)
    key = (cfg.N, cfg.IN, T_total, S_total,
           tuple((c.par, c.t0, len(c.tiles)) for c in chunks))
    if key not in _CACHE:
        _CACHE[key] = build_kernel(cfg, chunks, T_total, S_total)
    return _CACHE[key], in_maps


def run(cfg: Cfg, x, edge_index, Ws, bs, trace=False):
    nc, in_maps = _get_kernel(cfg, x, edge_index, Ws, bs)
    res = run_bass_kernel_spmd(nc, in_maps, list(range(cfg.C)), trace=trace)
    out = np.concatenate([res.results[k]["out"][:cfg.NL]
                          for k in range(cfg.C)], axis=0)
    return out, res


def kernel(x, edge_index, W1, b1, W2, b2, W3, b3, W4, b4):
    cfg = Cfg(N=x.shape[0], IN=x.shape[1])
    out, _ = run(cfg, np.asarray(x), np.asarray(edge_index),
                 [W1, W2, W3, W4], [b1, b2, b3, b4])
    return out.astype(np.float32)



# revision 5
# speedup vs baseline: 1.4537x; 1.1367x over previous
"""GCN (4-layer GCNConv) Trainium2 kernel — 8-core SPMD.

Strategy
--------
out = relu(^A (relu(^A (relu(^A (relu(^A X W1) W2)) W3))) W4)  with
^A = D^-1/2 (A + I) D^-1/2.  The symmetric norm dinv[src]*dinv[dst] is folded
into per-node row scalings, so the per-edge work is PURE data movement +
matmul-accumulate:

  h'_L = (dinv ⊙ o_{L-1}) @ W_L          (dense GEMM, node-sharded)
  agg[d] = sum_{e: dst(e)=d} h'_L[src(e)] (self-loops included as edges)
  o_L = relu(dinv ⊙ agg + b)             (per-node finalize)

Sharding: nodes (and their incoming edges) are split into 8 contiguous
buckets, one per NeuronCore.  Each layer: local GEMM -> AllGather of the
(fp16) feature rows -> per-core edge phase:

  * dma_gather (SWDGE) pulls h'[src] rows from HBM.  Rows are fp16 [64]
    = 128B which is below the 256B gather minimum, so indices address
    PAIRS of rows (idx = row>>1, elem 128 fp16 = 256B) and edges are
    grouped host-side by row parity; odd-parity groups gather from a
    64-element-shifted view so the payload is always elems [0:64].
  * edges are sorted by dst window (128 dst nodes); a one-hot fp16 matrix
    (built on DVE with a single is_equal tensor_tensor per chunk) maps each
    128-edge tile onto its dst slots, and the tensor engine accumulates
      psum[dst_slot, feat] += onehot.T @ msg
    into a per-window PSUM tile.  No scatter DMA, no per-edge vector math.

Host-side preprocessing (numpy, outside the timed kernel): degree/dinv
computation, bucketing + window sorting of edges, parity split, padding to
uniform shapes across cores (SPMD shares one program).
"""

import math
import os
import sys
from contextlib import ExitStack
from dataclasses import dataclass, field

import numpy as np

for _p in ("/opt/trn_rl_repo", "/root/.axon_site/_ro/trn_rl_repo"):
    if os.path.isdir(_p) and _p not in sys.path:
        sys.path.append(_p)

import concourse.bacc as bacc
import concourse.bass as bass
import concourse.tile as tile
from concourse import mybir
from concourse.bass_utils import run_bass_kernel_spmd

F32 = mybir.dt.float32
F16 = mybir.dt.float16
I16 = mybir.dt.int16

NPF16 = np.float16
SENTINEL = 255.0  # dst-in-window value for pad slots -> all-zero one-hot column


@dataclass
class Cfg:
    N: int = 50000
    IN: int = 128
    HID: int = 64
    C: int = 8
    WIN: int = 64        # dst window width (one-hot columns); 128 or 64
    SG: int = 2          # psum-pairs (128 dst rows) per gather super-group
    GEMM_CHUNK: int = 512

    @property
    def NL(self):
        return self.N // self.C

    @property
    def NWIN(self):       # pairs of 128 dst rows (psum tiles / staging cols)
        return (self.NL + 127) // 128

    @property
    def NWINW(self):      # one-hot windows
        return (self.NL + self.WIN - 1) // self.WIN

    @property
    def WPP(self):        # windows per psum pair-tile
        return 128 // self.WIN

    @property
    def NLP(self):
        return self.NWIN * 128

    @property
    def NROW(self):
        return self.C * self.NLP


@dataclass
class TileRec:
    w: int          # window within supergroup's window list (global window id)
    start: bool
    stop: bool


@dataclass
class ChunkRec:
    par: int
    t0: int         # first global tile index
    tiles: list = field(default_factory=list)  # list[TileRec]


def _build_layout(cfg: Cfg, counts: np.ndarray):
    """counts: [C, NWINW, 2] raw edge counts per (core, window, parity).
    Returns (chunks, T_total, S_total, group_caps[NWINW,2]) shared by all
    cores.  Supergroups cover SG psum-pairs (= SG*WPP windows) so a pair's
    windows always complete within one supergroup."""
    caps = counts.max(axis=0)  # [NWINW, 2]
    caps = ((caps + 127) // 128) * 128
    ntiles = caps // 128  # [NWINW, 2]

    chunks: list[ChunkRec] = []
    t = 0
    first_seen = set()
    last_par = {}
    for w in range(cfg.NWINW):
        if ntiles[w, 1] > 0:
            last_par[w] = 1
        elif ntiles[w, 0] > 0:
            last_par[w] = 0
    sgw = cfg.SG * cfg.WPP
    for sg0 in range(0, cfg.NWINW, sgw):
        ws = range(sg0, min(sg0 + sgw, cfg.NWINW))
        for par in (0, 1):
            ch = ChunkRec(par=par, t0=t)
            for w in ws:
                nt = int(ntiles[w, par])
                for j in range(nt):
                    start = w not in first_seen
                    first_seen.add(w)
                    stop = (par == last_par.get(w)) and (j == nt - 1)
                    ch.tiles.append(TileRec(w=w, start=start, stop=stop))
                    t += 1
            if ch.tiles:
                chunks.append(ch)
    T_total = t
    S_total = T_total * 128
    return chunks, T_total, S_total, caps


def host_prep(cfg: Cfg, x: np.ndarray, edge_index: np.ndarray,
              Ws: list, bs: list):
    """Returns (in_maps, chunks, T_total, S_total)."""
    N, NL, NLP, NWIN, C = cfg.N, cfg.NL, cfg.NLP, cfg.NWIN, cfg.C
    src = np.concatenate([np.asarray(edge_index[0], np.int64), np.arange(N)])
    dst = np.concatenate([np.asarray(edge_index[1], np.int64), np.arange(N)])
    deg = np.bincount(dst, minlength=N).astype(np.float32)
    dinv = (1.0 / np.sqrt(deg)).astype(np.float32)

    grow = (src // NL) * NLP + (src % NL)     # global padded row of each edge src
    par = (grow & 1).astype(np.int64)
    sidx = (grow >> 1).astype(np.int64)

    NWINW = cfg.NWINW
    bucket = dst // NL
    wloc = (dst % NL) // cfg.WIN
    dwin = (dst % NL) % cfg.WIN

    counts = np.zeros((C, NWINW, 2), np.int64)
    per_core = []
    for k in range(C):
        m = bucket == k
        g = wloc[m] * 2 + par[m]
        counts[k] = np.bincount(g, minlength=NWINW * 2).reshape(NWINW, 2)
        order = np.argsort(g, kind="stable")
        per_core.append((g[order], sidx[m][order], dwin[m][order]))

    chunks, T_total, S_total, caps = _build_layout(cfg, counts)

    # slot offsets per (w, par) group, following chunk order
    offs = np.zeros((NWINW, 2), np.int64)
    pos = 0
    sgw = cfg.SG * cfg.WPP
    for sg0 in range(0, NWINW, sgw):
        ws = range(sg0, min(sg0 + sgw, NWINW))
        for p in (0, 1):
            for w in ws:
                offs[w, p] = pos
                pos += caps[w, p]
    assert pos == S_total

    xp = np.asarray(x, np.float32) * dinv[:, None]

    in_maps = []
    for k in range(C):
        g, si, dw = per_core[k]
        gidx = np.zeros(S_total, np.int16)
        dstw = np.full(S_total, SENTINEL, NPF16)
        # starts of groups in the sorted per-core arrays
        gcnt = counts[k].reshape(-1)
        gstart = np.zeros(NWINW * 2, np.int64)
        gstart[1:] = np.cumsum(gcnt)[:-1]
        for w in range(NWINW):
            for p in (0, 1):
                n = int(counts[k, w, p])
                if n == 0:
                    continue
                s = int(gstart[w * 2 + p])
                o = int(offs[w, p])
                gidx[o:o + n] = si[s:s + n]
                dstw[o:o + n] = dw[s:s + n].astype(NPF16)
        gimg = np.tile(gidx.reshape(-1, 16).T, (8, 1))          # [128, S/16]
        dimg = dstw.reshape(T_total, 128).T.copy()              # [128, T]

        dinvw = np.zeros((128, NWIN), np.float32)
        for w in range(NWIN):
            nvalid = min(128, NL - w * 128)
            dinvw[:nvalid, w] = dinv[k * NL + w * 128: k * NL + w * 128 + nvalid]

        xT = np.zeros((cfg.IN, NLP), np.float32)
        xT[:, :NL] = xp[k * NL:(k + 1) * NL].T

        iota = np.broadcast_to(np.arange(128, dtype=NPF16), (128, 128)).copy()
        idf32 = np.eye(128, dtype=np.float32)

        im = {"xT": xT, "gidx": gimg, "dstw": dimg, "dinvw": dinvw,
              "iota": iota, "idf32": idf32}
        for i, (W, b) in enumerate(zip(Ws, bs), start=1):
            im[f"w{i}"] = np.asarray(W, np.float32)
            im[f"bb{i}"] = np.broadcast_to(np.asarray(b, np.float32),
                                           (128, cfg.HID)).copy()
        in_maps.append(im)
    return in_maps, chunks, T_total, S_total


def build_kernel(cfg: Cfg, chunks, T_total: int, S_total: int, debug=False):
    NL, NLP, NWIN, NROW, HID, IN = (cfg.NL, cfg.NLP, cfg.NWIN, cfg.NROW,
                                    cfg.HID, cfg.IN)
    nc = bacc.Bacc("TRN2", target_bir_lowering=False, debug=debug,
                   num_devices=cfg.C, num_swdge_queues=4)

    xT_d = nc.dram_tensor("xT", [IN, NLP], F32, kind="ExternalInput")
    gidx_d = nc.dram_tensor("gidx", [128, S_total // 16], I16, kind="ExternalInput")
    dstw_d = nc.dram_tensor("dstw", [128, T_total], F16, kind="ExternalInput")
    dinvw_d = nc.dram_tensor("dinvw", [128, NWIN], F32, kind="ExternalInput")
    iota_d = nc.dram_tensor("iota", [128, 128], F16, kind="ExternalInput")
    idf32_d = nc.dram_tensor("idf32", [128, 128], F32, kind="ExternalInput")
    w_d, bb_d = {}, {}
    for L in range(1, 5):
        KD = IN if L == 1 else HID
        w_d[L] = nc.dram_tensor(f"w{L}", [KD, HID], F32, kind="ExternalInput")
        bb_d[L] = nc.dram_tensor(f"bb{L}", [128, HID], F32, kind="ExternalInput")
    out_d = nc.dram_tensor("out", [NLP, HID], F32, kind="ExternalOutput")

    h_loc, h_full = {}, {}
    for L in range(1, 5):
        h_loc[L] = nc.dram_tensor(f"hloc{L}", [NLP, HID], F16)
        # +2 pad rows so the pair-packed odd-parity gather view never reads
        # past the tensor; they are zeroed once at startup.
        h_full[L] = nc.dram_tensor(f"hfull{L}", [NROW + 2, HID], F16,
                                   addr_space="Shared")

    groups = [list(range(cfg.C))]

    with tile.TileContext(nc) as tc, ExitStack() as ctx:
        cpool = ctx.enter_context(tc.tile_pool(name="consts", bufs=1))
        spool = ctx.enter_context(tc.tile_pool(name="stage", bufs=1))
        msg_pool = ctx.enter_context(tc.tile_pool(name="msg", bufs=5))
        oh_pool = ctx.enter_context(tc.tile_pool(name="oh", bufs=4))
        fin_pool = ctx.enter_context(tc.tile_pool(name="fin", bufs=4))
        ps_win = ctx.enter_context(tc.tile_pool(name="ps_win", bufs=4, space="PSUM"))
        ps_gemm = ctx.enter_context(tc.tile_pool(name="ps_gemm", bufs=2, space="PSUM"))
        ps_tr = ctx.enter_context(tc.tile_pool(name="ps_tr", bufs=2, space="PSUM"))
        # PSUM bank budget: ps_win 4 + ps_gemm 2 + ps_tr 2 (one shared tag) = 8

        def cload(dram, shape, dtype, tag):
            t = cpool.tile(shape, dtype, tag=tag)
            nc.sync.dma_start(t[:], dram[:])
            return t

        xT_s = cload(xT_d, [IN, NLP], F32, "xT")
        gidx_s = cload(gidx_d, [128, S_total // 16], I16, "gidx")
        dstw_s = cload(dstw_d, [128, T_total], F16, "dstw")
        dinvw_s = cload(dinvw_d, [128, NWIN], F32, "dinvw")
        iota_s = cload(iota_d, [128, 128], F16, "iota")
        idf32_s = cload(idf32_d, [128, 128], F32, "idf32")
        w_s = {L: cload(w_d[L], w_d[L].shape, F32, f"w{L}") for L in range(1, 5)}
        bb_s = {L: cload(bb_d[L], [128, HID], F32, f"bb{L}") for L in range(1, 5)}

        hT_s = spool.tile([HID, NLP], F32, tag="hT")
        oT_s = spool.tile([HID, NLP], F32, tag="oT")
        hstg = spool.tile([128, NWIN, HID], F16, tag="hstg")
        ostg = spool.tile([128, NWIN, HID], F32, tag="ostg")

        # zero the +2 trailing pad rows of every h_full (NaN-safety for the
        # pair-packed gather: junk*0 would still poison psum if junk is NaN)
        zrow = cpool.tile([2, HID], F16, tag="zrow")
        nc.vector.memset(zrow[:], 0)
        for L in range(1, 5):
            nc.sync.dma_start(h_full[L][NROW:NROW + 2], zrow[:])

        NC2 = NROW // 2
        abl_layers = int(os.environ.get("ABL_LAYERS", "4"))
        abl_nogather = bool(int(os.environ.get("ABL_NOGATHER", "0")))
        abl_nocc = bool(int(os.environ.get("ABL_NOCC", "0")))
        abl_repeat = int(os.environ.get("ABL_REPEAT", "1"))
        for L in [l for _ in range(abl_repeat)
                  for l in range(1, abl_layers + 1)]:
            KD = IN if L == 1 else HID
            movin = xT_s if L == 1 else oT_s

            # ---- GEMM: hT = W.T @ movin  -> [HID, NLP] (f32) ----
            for c0 in range(0, NLP, cfg.GEMM_CHUNK):
                cw = min(cfg.GEMM_CHUNK, NLP - c0)
                pg = ps_gemm.tile([HID, cfg.GEMM_CHUNK], F32, tag="pg")
                nc.tensor.matmul(pg[:, :cw], w_s[L][:KD, :], movin[:KD, c0:c0 + cw],
                                 start=True, stop=True)
                nc.scalar.copy(hT_s[:, c0:c0 + cw], pg[:, :cw])

            # ---- transpose hT -> fp16 rows, stage, store, allgather ----
            for w in range(NWIN):
                pt = ps_tr.tile([128, 128], F32, tag="pt")
                nc.tensor.transpose(pt[:, :HID],
                                    hT_s[:, w * 128:(w + 1) * 128],
                                    idf32_s[:HID, :HID])
                nc.scalar.copy(hstg[:, w, :], pt[:, :HID])
            nc.sync.dma_start(
                h_loc[L][:].rearrange("(w p) f -> p w f", p=128), hstg[:])
            if abl_nocc:
                nc.sync.dma_start(h_full[L][:NLP], h_loc[L][:])
            else:
                nc.gpsimd.collective_compute(
                    "AllGather", mybir.AluOpType.bypass, replica_groups=groups,
                    ins=[h_loc[L][:]], outs=[h_full[L][:NROW]])

            flat = h_full[L][:].flatten()
            view = [
                flat[0:NC2 * 128].rearrange("(n e) -> n e", e=128),
                flat[64:64 + NC2 * 128].rearrange("(n e) -> n e", e=128),
            ]

            # ---- edge phase ----
            WINW, WPP = cfg.WIN, cfg.WPP
            pair_nstop = {}
            for ch in chunks:
                for tl in ch.tiles:
                    if tl.stop:
                        p = (tl.w * WINW) // 128
                        pair_nstop[p] = pair_nstop.get(p, 0) + 1
            win_ps, pair_left = {}, {}
            for ci, ch in enumerate(chunks):
                nt = len(ch.tiles)
                msg = msg_pool.tile([128, nt, 128], F16, tag="msg")
                if abl_nogather:
                    nc.vector.memset(msg[:], 0)
                else:
                    # split across two SWDGE queues so two Q7 core-pairs
                    # generate descriptors for this chunk concurrently
                    nh = nt // 2
                    qa, qb = (2 * ci) % 4, (2 * ci + 1) % 4
                    if nh > 0:
                        nc.gpsimd.dma_gather(
                            msg[:, :nh, :], view[ch.par],
                            gidx_s[:, ch.t0 * 8:(ch.t0 + nh) * 8],
                            nh * 128, nh * 128, 128, single_packet=False,
                            queue_num=qa)
                    nc.gpsimd.dma_gather(
                        msg[:, nh:, :], view[ch.par],
                        gidx_s[:, (ch.t0 + nh) * 8:(ch.t0 + nt) * 8],
                        (nt - nh) * 128, (nt - nh) * 128, 128,
                        single_packet=False, queue_num=qb)
                oh = oh_pool.tile([128, nt, WINW], F16, tag="oh")
                nc.vector.tensor_tensor(
                    oh[:],
                    iota_s[:, :WINW].unsqueeze(1).broadcast_to((128, nt, WINW)),
                    dstw_s[:, ch.t0:ch.t0 + nt].unsqueeze(2)
                        .broadcast_to((128, nt, WINW)),
                    mybir.AluOpType.is_equal)
                for j, tl in enumerate(ch.tiles):
                    pair = (tl.w * WINW) // 128
                    soff = (tl.w * WINW) % 128
                    if pair not in win_ps:
                        win_ps[pair] = ps_win.tile([128, HID], F32, tag="pw",
                                                   name=f"pw{L}_{pair}")
                        pair_left[pair] = pair_nstop[pair]
                    nc.tensor.matmul(win_ps[pair][soff:soff + WINW, :],
                                     oh[:, j, :], msg[:, j, :HID],
                                     start=tl.start, stop=tl.stop,
                                     skip_group_check=True)
                    if tl.stop:
                        pair_left[pair] -= 1
                        if pair_left[pair] > 0:
                            continue
                        pw = win_ps.pop(pair)
                        dv = dinvw_s[:, pair:pair + 1]
                        if L < 4:
                            tmp = fin_pool.tile([128, HID], F32, tag="fin")
                            nc.vector.scalar_tensor_tensor(
                                tmp[:], pw[:], dv, bb_s[L][:],
                                mybir.AluOpType.mult, mybir.AluOpType.add)
                            nc.scalar.activation(
                                ostg[:, pair, :], tmp[:],
                                mybir.ActivationFunctionType.Relu, scale=dv)
                        else:
                            nc.vector.scalar_tensor_tensor(
                                ostg[:, pair, :], pw[:], dv, bb_s[L][:],
                                mybir.AluOpType.mult, mybir.AluOpType.add)

            if L < 4:
                # ---- oT = transpose(ostg) for next GEMM ----
                for w in range(NWIN):
                    pt = ps_tr.tile([128, 128], F32, tag="pt")
                    nc.tensor.transpose(pt[:HID, :],
                                        ostg[:, w, :], idf32_s[:])
                    nc.scalar.copy(oT_s[:, w * 128:(w + 1) * 128], pt[:HID, :])
            else:
                nc.sync.dma_start(
                    out_d[:].rearrange("(w p) f -> p w f", p=128), ostg[:])

    nc.compile()
    return nc


# ---------------------------------------------------------------------------

_CACHE = {}


def _get_kernel(cfg, x, edge_index, Ws, bs):
    in_maps, chunks, T_total, S_total = host_prep(cfg, x, edge_index, Ws, bs)
    key = (cfg.N, cfg.IN, T_total, S_total,
           tuple((c.par, c.t0, len(c.tiles)) for c in chunks))
    if key not in _CACHE:
        _CACHE[key] = build_kernel(cfg, chunks, T_total, S_total)
    return _CACHE[key], in_maps


def run(cfg: Cfg, x, edge_index, Ws, bs, trace=False):
    nc, in_maps = _get_kernel(cfg, x, edge_index, Ws, bs)
    res = run_bass_kernel_spmd(nc, in_maps, list(range(cfg.C)), trace=trace)
    out = np.concatenate([res.results[k]["out"][:cfg.NL]
                          for k in range(cfg.C)], axis=0)
    return out, res


def kernel(x, edge_index, W1, b1, W2, b2, W3, b3, W4, b4):
    cfg = Cfg(N=x.shape[0], IN=x.shape[1])
    out, _ = run(cfg, np.asarray(x), np.asarray(edge_index),
                 [W1, W2, W3, W4], [b1, b2, b3, b4])
    return out.astype(np.float32)

